# revision 1
# baseline (speedup 1.0000x reference)
"""Trainium2 Bass kernel for attention-weight computation.

Computes attn = softmax(encoder_outputs @ hidden) over seq_len=65536,
returning shape (1, 1, 65536) float32.

Distribution: encoder_outputs [65536, 1024] is sharded by rows across 8
NeuronCores (8192 rows each).  The host hands each core the TRANSPOSE of
its slice ([1024, 8192], h-major) so the contraction dim (h) lies on SBUF
partitions; the core streams it from HBM in 2 MiB tiles and computes its
8192 scores on the TensorEngine (hidden chunk = 1-column stationary
operand, E^T tile = [128, 512] moving operand, accumulating over the 8
h-chunks in [1, 512] PSUM tiles).

Softmax uses a single AllGather (flash-softmax style): each core reshapes
its scores to [16, 512] via a 32 KiB DRAM round-trip, computes
per-partition (max m_t, sum_t exp(s - m_t)) pairs, AllGathers the 8x32
pair vectors, locally combines them into the global max g and global sum
S, and rescales: attn = exp(s - m_t) * exp(m_t - g) / S.
"""

import numpy as np

S_TOTAL = 65536
H = 1024
N_CORES = 8
S_PER = S_TOTAL // N_CORES  # 8192 rows per core
P = 128                     # SBUF partitions
HC = H // P                 # 8 h-chunks
SEG = 512                   # segment width of the [N_SC, SEG] score layout
N_SC = S_PER // SEG         # 16 score segments

_CACHE: dict = {}


def _build_module(mm_dtype: str = "float32", variant: str = "small"):
    import concourse.bacc as bacc
    import concourse.mybir as mybir
    import concourse.tile as tile

    fp32 = mybir.dt.float32
    mmdt = getattr(mybir.dt, mm_dtype)
    AX = mybir.AxisListType.X
    ALL_CORES = [list(range(N_CORES))]
    Act = mybir.ActivationFunctionType

    nc = bacc.Bacc(
        "TRN2",
        target_bir_lowering=False,
        debug=False,
        enable_asserts=False,
        num_devices=N_CORES,
    )

    # et: transposed slice [H, S_PER]; hc: hidden as [P, HC] (chunk j in col j)
    et = nc.dram_tensor("et", [H, S_PER], mmdt, kind="ExternalInput").ap()
    hc = nc.dram_tensor("hc", [P, HC], mmdt, kind="ExternalInput").ap()
    out = nc.dram_tensor("out", [S_PER], fp32, kind="ExternalOutput").ap()

    with tile.TileContext(nc) as tc:
        with (
            tc.tile_pool(name="stream", bufs=4) as stream_pool,
            tc.tile_pool(name="persist", bufs=1) as persist_pool,
            tc.tile_pool(name="small", bufs=1) as small_pool,
            tc.tile_pool(name="psum", bufs=1, space="PSUM") as psum_pool,
            tc.tile_pool(name="dram", bufs=1, space="DRAM") as dram_pool,
        ):
            hid = small_pool.tile([P, HC], mmdt)
            nc.sync.dma_start(out=hid, in_=hc)
            ones = small_pool.tile([1, P], fp32)
            nc.vector.memset(ones, 1.0)

            # ---- scores: hidden chunk stationary, E^T moving ----
            # first chunks are small so the PE starts within a few us
            if variant == "big":
                sizes = [128, 128, 256, 512, 512, 512] + [1024] * 6
            else:
                sizes = [128, 128, 256] + [512] * 15
            assert sum(sizes) == S_PER
            et3 = et.rearrange("(j p) s -> p j s", p=P)
            scores_row = persist_pool.tile([1, S_PER], fp32)
            offs = [sum(sizes[:i]) for i in range(len(sizes))]

            # within each streamed chunk, interleave the <=512-wide PSUM
            # accumulation chains (distinct banks) so one chain's weight
            # loads hide under the other's matmuls; no cross-chunk
            # coupling of the in-order PE queue to a later DMA
            stream_bufs = 8 if max(sizes) <= 512 else 4
            for c, size in enumerate(sizes):
                etile = stream_pool.tile(
                    [P, HC, size], mmdt, tag="et", bufs=stream_bufs, name=f"et{c}"
                )
                nc.sync.dma_start(
                    out=etile, in_=et3[:, :, offs[c] : offs[c] + size]
                )
                nsub = max(1, size // 512)
                sub = size // nsub
                pss = [
                    psum_pool.tile([1, sub], fp32, tag="ps", bufs=6, name=f"ps{c}_{u}")
                    for u in range(nsub)
                ]
                for j in range(HC):
                    for u in range(nsub):
                        nc.tensor.matmul(
                            pss[u],
                            hid[:, j : j + 1],
                            etile[:, j, u * sub : (u + 1) * sub],
                            start=(j == 0),
                            stop=(j == HC - 1),
                        )
                for u in range(nsub):
                    nc.scalar.copy(
                        scores_row[
                            :, offs[c] + u * sub : offs[c] + (u + 1) * sub
                        ],
                        pss[u],
                    )
            # ---- reshape scores [1, 8192] -> [16, 512] via DRAM ----
            sc_dram = dram_pool.tile([S_PER], fp32)
            nc.scalar.dma_start(out=sc_dram, in_=scores_row)
            scores16 = persist_pool.tile([N_SC, SEG], fp32)
            nc.scalar.dma_start(
                out=scores16, in_=sc_dram.rearrange("(t n) -> t n", t=N_SC)
            )

            # ---- local flash-softmax stats ----
            pair16 = small_pool.tile([N_SC, 2], fp32)  # [:,0]=max, [:,1]=sumexp
            nc.vector.reduce_max(pair16[:, 0:1], scores16, axis=AX)
            negm16 = small_pool.tile([N_SC, 1], fp32)
            nc.vector.tensor_scalar_mul(negm16, pair16[:, 0:1], -1.0)
            exps16 = persist_pool.tile([N_SC, SEG], fp32)
            nc.scalar.activation(
                out=exps16,
                in_=scores16,
                func=Act.Exp,
                bias=negm16,
                scale=1.0,
                accum_out=pair16[:, 1:2],
            )

            # ---- one AllGather of the (m, S) pairs ----
            cc_in = dram_pool.tile([N_SC * 2], fp32)
            cc_out = dram_pool.tile([N_CORES, N_SC * 2], fp32)
            nc.scalar.dma_start(out=cc_in, in_=pair16)
            nc.gpsimd.collective_compute(
                "AllGather",
                mybir.AluOpType.bypass,
                replica_groups=ALL_CORES,
                ins=[cc_in.opt()],
                outs=[cc_out.opt()],
            )

            # ---- combine: g = max m; S = sum s*exp(m-g); factors ----
            row = small_pool.tile([1, N_CORES * N_SC * 2], fp32)
            nc.scalar.dma_start(out=row, in_=cc_out.rearrange("a b -> (a b)"))
            rowv = row.rearrange("o (k two) -> o two k", two=2)
            g1 = small_pool.tile([1, 1], fp32)
            nc.vector.reduce_max(g1, rowv[:, 0, :], axis=AX)
            negg1 = small_pool.tile([1, 1], fp32)
            nc.vector.tensor_scalar_mul(negg1, g1, -1.0)
            em = small_pool.tile([1, N_CORES * N_SC], fp32)
            nc.scalar.activation(
                out=em, in_=rowv[:, 0, :], func=Act.Exp, bias=negg1, scale=1.0
            )
            terms = small_pool.tile([1, N_CORES * N_SC], fp32)
            nc.vector.tensor_mul(terms, em, rowv[:, 1, :])
            s1 = small_pool.tile([1, 1], fp32)
            nc.vector.reduce_sum(s1, terms, axis=AX)
            rs1 = small_pool.tile([1, 1], fp32)
            nc.vector.reciprocal(rs1, s1)

            # broadcast (-g, 1/S) to the 16 partitions via ones.T @ pack
            pack = small_pool.tile([1, 2], fp32)
            nc.vector.tensor_copy(pack[:, 0:1], negg1)
            nc.vector.tensor_copy(pack[:, 1:2], rs1)
            bpsum = psum_pool.tile([N_SC, 2], fp32)
            nc.tensor.matmul(bpsum, ones[:, 0:N_SC], pack, start=True, stop=True)
            bsc = small_pool.tile([N_SC, 2], fp32)
            nc.scalar.copy(bsc, bpsum)

            # f = exp(m - g); attn = exps * f * (1/S)
            f16 = small_pool.tile([N_SC, 1], fp32)
            nc.scalar.activation(
                out=f16, in_=pair16[:, 0:1], func=Act.Exp, bias=bsc[:, 0:1], scale=1.0
            )
            attn16 = persist_pool.tile([N_SC, SEG], fp32)
            nc.vector.tensor_scalar(
                out=attn16,
                in0=exps16,
                scalar1=f16,
                scalar2=bsc[:, 1:2],
                op0=mybir.AluOpType.mult,
                op1=mybir.AluOpType.mult,
            )
            nc.sync.dma_start(
                out=out.rearrange("(t n) -> t n", t=N_SC), in_=attn16
            )

    nc.compile()
    return nc


def _get_module():
    if "nc" not in _CACHE:
        _CACHE["nc"] = _build_module()
    return _CACHE["nc"]


def _prep_inputs(hidden: np.ndarray, encoder_outputs: np.ndarray):
    hidden = np.asarray(hidden, dtype=np.float32)
    eo = np.asarray(encoder_outputs, dtype=np.float32)
    hcm = np.ascontiguousarray(hidden.reshape(HC, P).T)  # [P, HC]
    in_maps = []
    for c in range(N_CORES):
        ets = np.ascontiguousarray(eo[c * S_PER : (c + 1) * S_PER].T)  # [H, S_PER]
        in_maps.append({"et": ets, "hc": hcm})
    return in_maps


def _run(hidden: np.ndarray, encoder_outputs: np.ndarray, trace: bool = False):
    from concourse.bass_utils import run_bass_kernel_spmd

    nc = _get_module()
    in_maps = _prep_inputs(hidden, encoder_outputs)
    res = run_bass_kernel_spmd(
        nc, in_maps, core_ids=list(range(N_CORES)), trace=trace
    )
    parts = [np.asarray(res.results[c]["out"]).reshape(-1) for c in range(N_CORES)]
    attn = np.concatenate(parts)
    return attn.reshape(1, 1, S_TOTAL).astype(np.float32), res


def kernel(hidden: np.ndarray, encoder_outputs: np.ndarray) -> np.ndarray:
    try:
        out, _ = _run(hidden, encoder_outputs, trace=False)
    except Exception:
        # one retry for transient device/runtime hiccups
        _CACHE.clear()
        out, _ = _run(hidden, encoder_outputs, trace=False)
    return out



# revision 2
# speedup vs baseline: 1.4162x; 1.4162x over previous
"""Trainium2 Bass kernel for attention-weight computation.

Computes attn = softmax(encoder_outputs @ hidden) over seq_len=65536,
returning shape (1, 1, 65536) float32.

Distribution: encoder_outputs [65536, 1024] is sharded by rows across 8
NeuronCores (8192 rows each).  The host casts each slice to fp16 (accuracy
checked: rel_l2 ~3e-5 vs fp32 reference, far under the 2e-2 gate — softmax
normalization cancels the dominant entry's quantization error) and
pre-tiles it so every DMA reads fully contiguous 8 KiB partition lines:
per chunk c of `size` seq columns, the host stores [128, HC*size] fp16
where partition p holds h-chunk-major data etile[p, j*size+s] =
E[base+s, j*128+p].  The core streams these chunks from HBM and computes
its 8192 scores on the TensorEngine (hidden chunk = 1-column stationary
operand, chunk tile = [128, <=512] moving operand, accumulating the 8
h-chunks into a [1, size] PSUM tile).

Softmax stats are computed incrementally during the stream (per-chunk max
+ sum-of-exp directly from PSUM on the otherwise-idle Vector/Act
engines).  One AllGather of the first 17 chunks' (m, s) pairs overlaps
the stream tail; a second tiny AllGather covers the last (small) chunk.
The tail combines the 144 gathered pairs into the global max g and sum S,
then rescales the stored exp(s - m_t) rows by exp(m_t - g)/S split across
the Vector and Act engines, and writes the result out.
"""

import numpy as np

S_TOTAL = 65536
H = 1024
N_CORES = 8
S_PER = S_TOTAL // N_CORES  # 8192 rows per core
P = 128                     # SBUF partitions
HC = H // P                 # 8 h-chunks

# chunk sizes: small first chunks for pipeline ramp, small last chunk so
# the final stats (which gate the tail AllGather) are ready quickly
SIZES = [128, 256] + [512] * 15 + [128]
assert sum(SIZES) == S_PER
OFFS = [sum(SIZES[:i]) for i in range(len(SIZES))]
NCH = len(SIZES)            # 18 chunks
AG1_CH = NCH - 1            # chunks [0, AG1_CH) go in the overlapped AllGather

_CACHE: dict = {}


def _build_module(mm_dtype: str = "float16"):
    import concourse.bacc as bacc
    import concourse.mybir as mybir
    import concourse.tile as tile

    fp32 = mybir.dt.float32
    mmdt = getattr(mybir.dt, mm_dtype)
    AX = mybir.AxisListType.X
    ALL_CORES = [list(range(N_CORES))]
    Act = mybir.ActivationFunctionType

    nc = bacc.Bacc(
        "TRN2",
        target_bir_lowering=False,
        debug=False,
        enable_asserts=False,
        num_devices=N_CORES,
    )

    # et: pre-tiled slice [P, HC*S_PER]; hc: hidden as [P, HC] (chunk j in col j)
    et = nc.dram_tensor("et", [P, HC * S_PER], mmdt, kind="ExternalInput").ap()
    hc = nc.dram_tensor("hc", [P, HC], mmdt, kind="ExternalInput").ap()
    out = nc.dram_tensor("out", [S_PER], fp32, kind="ExternalOutput").ap()

    with tile.TileContext(nc) as tc:
        with (
            tc.tile_pool(name="stream", bufs=6) as stream_pool,
            tc.tile_pool(name="persist", bufs=1) as persist_pool,
            tc.tile_pool(name="small", bufs=1) as small_pool,
            tc.tile_pool(name="psum", bufs=6, space="PSUM") as psum_pool,
            tc.tile_pool(name="dram", bufs=1, space="DRAM") as dram_pool,
        ):
            hid = small_pool.tile([P, HC], mmdt)
            nc.sync.dma_start(out=hid, in_=hc)

            exps_row = persist_pool.tile([1, S_PER], fp32)   # exp(s - m_t)
            attn_row = persist_pool.tile([1, S_PER], fp32)
            pair_row = small_pool.tile([1, 2 * NCH], fp32)   # (m_t, sum_t)
            negm_row = small_pool.tile([1, NCH], fp32)

            cc_in1 = dram_pool.tile([2 * AG1_CH], fp32)
            cc_out1 = dram_pool.tile([N_CORES, 2 * AG1_CH], fp32)
            cc_in2 = dram_pool.tile([2 * (NCH - AG1_CH)], fp32)
            cc_out2 = dram_pool.tile([N_CORES, 2 * (NCH - AG1_CH)], fp32)

            # ---- stream chunks: matmul + incremental softmax stats ----
            for c, size in enumerate(SIZES):
                etile = stream_pool.tile(
                    [P, HC * size], mmdt, tag="et", bufs=6, name=f"et{c}"
                )
                eng = nc.sync if c % 2 == 0 else nc.scalar
                eng.dma_start(
                    out=etile,
                    in_=et[:, HC * OFFS[c] : HC * (OFFS[c] + size)],
                )
                ps = psum_pool.tile([1, size], fp32, tag="ps", bufs=6, name=f"ps{c}")
                for j in range(HC):
                    nc.tensor.matmul(
                        ps,
                        hid[:, j : j + 1],
                        etile[:, j * size : (j + 1) * size],
                        start=(j == 0),
                        stop=(j == HC - 1),
                    )
                # stats straight from PSUM; ACT writes exp row + running sum
                nc.vector.reduce_max(pair_row[:, 2 * c : 2 * c + 1], ps, axis=AX)
                nc.vector.tensor_scalar_mul(
                    negm_row[:, c : c + 1], pair_row[:, 2 * c : 2 * c + 1], -1.0
                )
                nc.scalar.activation(
                    out=exps_row[:, OFFS[c] : OFFS[c] + size],
                    in_=ps,
                    func=Act.Exp,
                    bias=negm_row[:, c : c + 1],
                    scale=1.0,
                    accum_out=pair_row[:, 2 * c + 1 : 2 * c + 2],
                )
                if c == AG1_CH - 1:
                    # overlap the big AllGather with the remaining stream
                    nc.scalar.dma_start(out=cc_in1, in_=pair_row[:, 0 : 2 * AG1_CH])
                    nc.gpsimd.collective_compute(
                        "AllGather",
                        mybir.AluOpType.bypass,
                        replica_groups=ALL_CORES,
                        ins=[cc_in1.opt()],
                        outs=[cc_out1.opt()],
                    )

            # ---- tail AllGather for the last chunk's stats ----
            nc.scalar.dma_start(out=cc_in2, in_=pair_row[:, 2 * AG1_CH : 2 * NCH])
            nc.gpsimd.collective_compute(
                "AllGather",
                mybir.AluOpType.bypass,
                replica_groups=ALL_CORES,
                ins=[cc_in2.opt()],
                outs=[cc_out2.opt()],
            )

            # ---- combine 144 (m, s) pairs -> g, S ----
            NP1 = N_CORES * 2 * AG1_CH          # 272 floats from AG1
            NP2 = N_CORES * 2 * (NCH - AG1_CH)  # 16 floats from AG2
            row = small_pool.tile([1, NP1 + NP2], fp32)
            nc.scalar.dma_start(out=row[:, 0:NP1], in_=cc_out1.rearrange("a b -> (a b)"))
            nc.scalar.dma_start(
                out=row[:, NP1 : NP1 + NP2], in_=cc_out2.rearrange("a b -> (a b)")
            )
            rowv = row.rearrange("o (k two) -> o two k", two=2)
            g1 = small_pool.tile([1, 1], fp32)
            nc.vector.reduce_max(g1, rowv[:, 0, :], axis=AX)
            negg1 = small_pool.tile([1, 1], fp32)
            nc.vector.tensor_scalar_mul(negg1, g1, -1.0)
            em = small_pool.tile([1, (NP1 + NP2) // 2], fp32)
            nc.scalar.activation(
                out=em, in_=rowv[:, 0, :], func=Act.Exp, bias=negg1, scale=1.0
            )
            terms = small_pool.tile([1, (NP1 + NP2) // 2], fp32)
            nc.vector.tensor_mul(terms, em, rowv[:, 1, :])
            s1 = small_pool.tile([1, 1], fp32)
            nc.vector.reduce_sum(s1, terms, axis=AX)
            rs1 = small_pool.tile([1, 1], fp32)
            nc.vector.reciprocal(rs1, s1)

            # my per-chunk factors f_t = exp(m_t - g) / S
            pairv = pair_row.rearrange("o (k two) -> o two k", two=2)
            ft0 = small_pool.tile([1, NCH], fp32)
            nc.scalar.activation(
                out=ft0, in_=pairv[:, 0, :], func=Act.Exp, bias=negg1, scale=1.0
            )
            ft = small_pool.tile([1, NCH], fp32)
            nc.vector.tensor_scalar_mul(ft, ft0, rs1)

            # ---- rescale, split across Vector and Act engines ----
            for c, size in enumerate(SIZES):
                src = exps_row[:, OFFS[c] : OFFS[c] + size]
                dst = attn_row[:, OFFS[c] : OFFS[c] + size]
                if c % 2 == 0:
                    nc.vector.tensor_scalar_mul(dst, src, ft[:, c : c + 1])
                else:
                    nc.scalar.mul(dst, src, ft[:, c : c + 1])

            half = S_PER // 2
            out2 = out.rearrange("(o s) -> o s", o=1)
            nc.sync.dma_start(out=out2[:, 0:half], in_=attn_row[:, 0:half])
            nc.scalar.dma_start(out=out2[:, half:], in_=attn_row[:, half:])

    nc.compile()
    return nc


def _get_module():
    if "nc" not in _CACHE:
        _CACHE["nc"] = _build_module()
    return _CACHE["nc"]


def _prep_inputs(hidden: np.ndarray, encoder_outputs: np.ndarray):
    hidden = np.asarray(hidden, dtype=np.float32)
    eo = np.asarray(encoder_outputs, dtype=np.float32)
    h16 = hidden.astype(np.float16)
    eo16 = eo.astype(np.float16)
    hcm = np.ascontiguousarray(h16.reshape(HC, P).T)  # [P, HC]
    in_maps = []
    for c in range(N_CORES):
        es = eo16[c * S_PER : (c + 1) * S_PER]  # [S_PER, H]
        blocks = [
            es[OFFS[i] : OFFS[i] + SIZES[i], :]
            .reshape(SIZES[i], HC, P)
            .transpose(2, 1, 0)
            .reshape(P, HC * SIZES[i])
            for i in range(NCH)
        ]
        ets = np.ascontiguousarray(np.concatenate(blocks, axis=1))  # [P, HC*S_PER]
        in_maps.append({"et": ets, "hc": hcm})
    return in_maps


def _run(hidden: np.ndarray, encoder_outputs: np.ndarray, trace: bool = False):
    from concourse.bass_utils import run_bass_kernel_spmd

    nc = _get_module()
    in_maps = _prep_inputs(hidden, encoder_outputs)
    res = run_bass_kernel_spmd(
        nc, in_maps, core_ids=list(range(N_CORES)), trace=trace
    )
    parts = [np.asarray(res.results[c]["out"]).reshape(-1) for c in range(N_CORES)]
    attn = np.concatenate(parts)
    return attn.reshape(1, 1, S_TOTAL).astype(np.float32), res


def kernel(hidden: np.ndarray, encoder_outputs: np.ndarray) -> np.ndarray:
    try:
        out, _ = _run(hidden, encoder_outputs, trace=False)
    except Exception:
        # one retry for transient device/runtime hiccups
        _CACHE.clear()
        out, _ = _run(hidden, encoder_outputs, trace=False)
    return out


# revision 6
# speedup vs baseline: 1.7346x; 1.2248x over previous
"""Trainium2 Bass kernel for attention-weight computation.

Computes attn = softmax(encoder_outputs @ hidden) over seq_len=65536,
returning shape (1, 1, 65536) float32.

Distribution: encoder_outputs [65536, 1024] is sharded by rows across 8
NeuronCores (8192 rows each).  The host casts each slice to fp16 (accuracy
checked: rel_l2 ~3e-5 vs fp32 reference, far under the 2e-2 gate — softmax
normalization cancels the dominant entry's quantization error) and
pre-tiles it so every DMA reads fully contiguous 8 KiB partition lines:
per chunk c of `size` seq columns, the host stores [128, HC*size] fp16
where partition p holds h-chunk-major data etile[p, j*size+s] =
E[base+s, j*128+p].  The core streams these chunks from HBM and computes
its 8192 scores on the TensorEngine (hidden chunk = 1-column stationary
operand, chunk tile = [128, <=512] moving operand, accumulating the 8
h-chunks into a [1, size] PSUM tile).

Softmax stats are computed incrementally during the stream (per-chunk max
+ sum-of-exp directly from PSUM on the otherwise-idle Vector/Act
engines).  One AllGather of the first 17 chunks' (m, s) pairs overlaps
the stream tail; a second tiny AllGather covers the last (small) chunk.
The tail combines the 144 gathered pairs into the global max g and sum S,
then rescales the stored exp(s - m_t) rows by exp(m_t - g)/S split across
the Vector and Act engines, and writes the result out.
"""

import numpy as np

S_TOTAL = 65536
H = 1024
N_CORES = 8
S_PER = S_TOTAL // N_CORES  # 8192 rows per core
P = 128                     # SBUF partitions
HC = H // P                 # 8 h-chunks

# chunk sizes: small first chunks for pipeline ramp, small last chunk so
# the final stats (which gate the tail AllGather) are ready quickly
SIZES = [128, 256] + [512] * 15 + [128]
assert sum(SIZES) == S_PER
OFFS = [sum(SIZES[:i]) for i in range(len(SIZES))]
NCH = len(SIZES)            # 18 chunks
AG1_CH = 10                 # chunks [0, AG1_CH) go in the mid-stream AllGather

_CACHE: dict = {}


def _build_module(mm_dtype: str = "float16"):
    import concourse.bacc as bacc
    import concourse.mybir as mybir
    import concourse.tile as tile

    fp32 = mybir.dt.float32
    mmdt = getattr(mybir.dt, mm_dtype)
    AX = mybir.AxisListType.X
    ALL_CORES = [list(range(N_CORES))]
    Act = mybir.ActivationFunctionType

    nc = bacc.Bacc(
        "TRN2",
        target_bir_lowering=False,
        debug=False,
        enable_asserts=False,
        num_devices=N_CORES,
    )

    # et: pre-tiled slice [P, HC*S_PER]; hc: hidden as [P, HC] (chunk j in col j)
    et = nc.dram_tensor("et", [P, HC * S_PER], mmdt, kind="ExternalInput").ap()
    hc = nc.dram_tensor("hc", [P, HC], mmdt, kind="ExternalInput").ap()
    out = nc.dram_tensor("out", [S_PER], fp32, kind="ExternalOutput").ap()

    with tile.TileContext(nc) as tc:
        with (
            tc.tile_pool(name="stream", bufs=6) as stream_pool,
            tc.tile_pool(name="persist", bufs=1) as persist_pool,
            tc.tile_pool(name="small", bufs=1) as small_pool,
            tc.tile_pool(name="psum", bufs=6, space="PSUM") as psum_pool,
            tc.tile_pool(name="dram", bufs=1, space="DRAM") as dram_pool,
        ):
            hid = small_pool.tile([P, HC], mmdt)
            nc.sync.dma_start(out=hid, in_=hc)

            # tiny warmup collective: absorbs cross-core launch skew and
            # the pre-collective barrier early, fully overlapped with the
            # stream (collectives run on TOPSP/SDMA, not our engines)
            warm = small_pool.tile([1, 1], fp32)
            nc.vector.memset(warm, 0.0)
            cc_warm_in = dram_pool.tile([1], fp32)
            cc_warm_out = dram_pool.tile([N_CORES], fp32)
            nc.gpsimd.dma_start(out=cc_warm_in, in_=warm)
            nc.gpsimd.collective_compute(
                "AllGather",
                mybir.AluOpType.bypass,
                replica_groups=ALL_CORES,
                ins=[cc_warm_in.opt()],
                outs=[cc_warm_out.opt()],
            )

            exps_row = persist_pool.tile([1, S_PER], fp32)   # exp(s - m_t)
            attn_row = persist_pool.tile([1, S_PER], fp32)
            pair_row = small_pool.tile([1, 2 * NCH], fp32)   # (m_t, sum_t)
            negm_row = small_pool.tile([1, NCH], fp32)

            cc_in1 = dram_pool.tile([2 * AG1_CH], fp32)
            cc_out1 = dram_pool.tile([N_CORES, 2 * AG1_CH], fp32)
            cc_in2 = dram_pool.tile([2 * (NCH - AG1_CH)], fp32)
            cc_out2 = dram_pool.tile([N_CORES, 2 * (NCH - AG1_CH)], fp32)

            # ---- stream chunks: matmul + incremental softmax stats ----
            for c, size in enumerate(SIZES):
                etile = stream_pool.tile(
                    [P, HC * size], mmdt, tag="et", bufs=6, name=f"et{c}"
                )
                eng = nc.sync if c % 2 == 0 else nc.scalar
                eng.dma_start(
                    out=etile,
                    in_=et[:, HC * OFFS[c] : HC * (OFFS[c] + size)],
                )
                ps = psum_pool.tile([1, size], fp32, tag="ps", bufs=6, name=f"ps{c}")
                for j in range(HC):
                    nc.tensor.matmul(
                        ps,
                        hid[:, j : j + 1],
                        etile[:, j * size : (j + 1) * size],
                        start=(j == 0),
                        stop=(j == HC - 1),
                    )
                # stats straight from PSUM; ACT writes exp row + running sum
                nc.vector.reduce_max(pair_row[:, 2 * c : 2 * c + 1], ps, axis=AX)
                nc.vector.tensor_scalar_mul(
                    negm_row[:, c : c + 1], pair_row[:, 2 * c : 2 * c + 1], -1.0
                )
                nc.scalar.activation(
                    out=exps_row[:, OFFS[c] : OFFS[c] + size],
                    in_=ps,
                    func=Act.Exp,
                    bias=negm_row[:, c : c + 1],
                    scale=1.0,
                    accum_out=pair_row[:, 2 * c + 1 : 2 * c + 2],
                )
                if c == AG1_CH - 1:
                    # overlap the big AllGather with the remaining stream;
                    # SWDGE (gpsimd) DMA so it doesn't queue behind the
                    # stream DMAs on the HWDGE rings
                    nc.gpsimd.dma_start(out=cc_in1, in_=pair_row[:, 0 : 2 * AG1_CH])
                    nc.gpsimd.collective_compute(
                        "AllGather",
                        mybir.AluOpType.bypass,
                        replica_groups=ALL_CORES,
                        ins=[cc_in1.opt()],
                        outs=[cc_out1.opt()],
                    )

            # ---- tail AllGather for the remaining chunks' stats ----
            nc.gpsimd.dma_start(out=cc_in2, in_=pair_row[:, 2 * AG1_CH : 2 * NCH])
            nc.gpsimd.collective_compute(
                "AllGather",
                mybir.AluOpType.bypass,
                replica_groups=ALL_CORES,
                ins=[cc_in2.opt()],
                outs=[cc_out2.opt()],
            )

            # ---- combine 144 (m, s) pairs -> g, S ----
            NP1 = N_CORES * 2 * AG1_CH          # 272 floats from AG1
            NP2 = N_CORES * 2 * (NCH - AG1_CH)  # 16 floats from AG2
            row = small_pool.tile([1, NP1 + NP2], fp32)
            nc.scalar.dma_start(out=row[:, 0:NP1], in_=cc_out1.rearrange("a b -> (a b)"))
            nc.scalar.dma_start(
                out=row[:, NP1 : NP1 + NP2], in_=cc_out2.rearrange("a b -> (a b)")
            )
            rowv = row.rearrange("o (k two) -> o two k", two=2)
            g1 = small_pool.tile([1, 1], fp32)
            nc.vector.reduce_max(g1, rowv[:, 0, :], axis=AX)
            negg1 = small_pool.tile([1, 1], fp32)
            nc.vector.tensor_scalar_mul(negg1, g1, -1.0)
            em = small_pool.tile([1, (NP1 + NP2) // 2], fp32)
            nc.scalar.activation(
                out=em, in_=rowv[:, 0, :], func=Act.Exp, bias=negg1, scale=1.0
            )
            terms = small_pool.tile([1, (NP1 + NP2) // 2], fp32)
            nc.vector.tensor_mul(terms, em, rowv[:, 1, :])
            s1 = small_pool.tile([1, 1], fp32)
            nc.vector.reduce_sum(s1, terms, axis=AX)
            rs1 = small_pool.tile([1, 1], fp32)
            nc.vector.reciprocal(rs1, s1)

            # my per-chunk factors f_t = exp(m_t - g) / S
            pairv = pair_row.rearrange("o (k two) -> o two k", two=2)
            ft0 = small_pool.tile([1, NCH], fp32)
            nc.scalar.activation(
                out=ft0, in_=pairv[:, 0, :], func=Act.Exp, bias=negg1, scale=1.0
            )
            ft = small_pool.tile([1, NCH], fp32)
            nc.vector.tensor_scalar_mul(ft, ft0, rs1)

            # ---- rescale, split across Vector and Act engines ----
            for c, size in enumerate(SIZES):
                src = exps_row[:, OFFS[c] : OFFS[c] + size]
                dst = attn_row[:, OFFS[c] : OFFS[c] + size]
                if c % 2 == 0:
                    nc.vector.tensor_scalar_mul(dst, src, ft[:, c : c + 1])
                else:
                    nc.scalar.mul(dst, src, ft[:, c : c + 1])

            half = S_PER // 2
            out2 = out.rearrange("(o s) -> o s", o=1)
            nc.sync.dma_start(out=out2[:, 0:half], in_=attn_row[:, 0:half])
            nc.scalar.dma_start(out=out2[:, half:], in_=attn_row[:, half:])

    nc.compile()
    return nc


def _get_module():
    if "nc" not in _CACHE:
        _CACHE["nc"] = _build_module()
    return _CACHE["nc"]


def _prep_inputs(hidden: np.ndarray, encoder_outputs: np.ndarray):
    hidden = np.asarray(hidden, dtype=np.float32)
    eo = np.asarray(encoder_outputs, dtype=np.float32)
    h16 = hidden.astype(np.float16)
    eo16 = eo.astype(np.float16)
    hcm = np.ascontiguousarray(h16.reshape(HC, P).T)  # [P, HC]
    in_maps = []
    for c in range(N_CORES):
        es = eo16[c * S_PER : (c + 1) * S_PER]  # [S_PER, H]
        blocks = [
            es[OFFS[i] : OFFS[i] + SIZES[i], :]
            .reshape(SIZES[i], HC, P)
            .transpose(2, 1, 0)
            .reshape(P, HC * SIZES[i])
            for i in range(NCH)
        ]
        ets = np.ascontiguousarray(np.concatenate(blocks, axis=1))  # [P, HC*S_PER]
        in_maps.append({"et": ets, "hc": hcm})
    return in_maps


def _run(hidden: np.ndarray, encoder_outputs: np.ndarray, trace: bool = False):
    from concourse.bass_utils import run_bass_kernel_spmd

    nc = _get_module()
    in_maps = _prep_inputs(hidden, encoder_outputs)
    res = run_bass_kernel_spmd(
        nc, in_maps, core_ids=list(range(N_CORES)), trace=trace
    )
    parts = [np.asarray(res.results[c]["out"]).reshape(-1) for c in range(N_CORES)]
    attn = np.concatenate(parts)
    return attn.reshape(1, 1, S_TOTAL).astype(np.float32), res


def kernel(hidden: np.ndarray, encoder_outputs: np.ndarray) -> np.ndarray:
    try:
        out, _ = _run(hidden, encoder_outputs, trace=False)
    except Exception:
        # one retry for transient device/runtime hiccups
        _CACHE.clear()
        out, _ = _run(hidden, encoder_outputs, trace=False)
    return out


# revision 9
# speedup vs baseline: 1.9496x; 1.1240x over previous
"""Trainium2 Bass kernel for attention-weight computation.

Computes attn = softmax(encoder_outputs @ hidden) over seq_len=65536,
returning shape (1, 1, 65536) float32.

Distribution: encoder_outputs [65536, 1024] is sharded by rows across 8
NeuronCores (8192 rows each).  The host casts each slice to fp16 (accuracy
checked: rel_l2 ~3e-5 vs fp32 reference, far under the 2e-2 gate — softmax
normalization cancels the dominant entry's quantization error) and
pre-tiles it so every DMA reads fully contiguous 8 KiB partition lines:
per chunk c of `size` seq columns, the host stores [128, HC*size] fp16
where partition p holds h-chunk-major data etile[p, j*size+s] =
E[base+s, j*128+p].  The core streams these chunks from HBM and computes
its 8192 scores on the TensorEngine (hidden chunk = 1-column stationary
operand, chunk tile = [128, <=512] moving operand, accumulating the 8
h-chunks into a [1, size] PSUM tile).

Softmax stats are computed incrementally during the stream (per-chunk max
+ sum-of-exp directly from PSUM on the otherwise-idle Vector/Act
engines).  One AllGather of the first 17 chunks' (m, s) pairs overlaps
the stream tail; a second tiny AllGather covers the last (small) chunk.
The tail combines the 144 gathered pairs into the global max g and sum S,
then rescales the stored exp(s - m_t) rows by exp(m_t - g)/S split across
the Vector and Act engines, and writes the result out.
"""

import numpy as np

S_TOTAL = 65536
H = 1024
N_CORES = 8
S_PER = S_TOTAL // N_CORES  # 8192 rows per core
P = 128                     # SBUF partitions
HC = H // P                 # 8 h-chunks

# chunk sizes: small first chunks for pipeline ramp, small last chunk so
# the final stats (which gate the tail AllGather) are ready quickly.
# 512-col subchunks are processed in pairs sharing the h-loop (two PSUM
# accumulation chains) so consecutive matmuls hit different banks and
# pipeline instead of paying the isolated-matmul drain each time.
SIZES = [128, 256] + [512] * 15 + [128]
assert sum(SIZES) == S_PER
OFFS = [sum(SIZES[:i]) for i in range(len(SIZES))]
NCH = len(SIZES)            # 18 chunks
# DMA groups: chunk indices loaded in one dma_start (pairs of 512s)
DMA_GROUPS = [[0], [1], [2, 3], [4, 5], [6, 7], [8, 9], [10, 11], [12, 13],
              [14, 15], [16], [17]]
assert sorted(c for g in DMA_GROUPS for c in g) == list(range(NCH))

_CACHE: dict = {}


def _build_module(mm_dtype: str = "float16"):
    import concourse.bacc as bacc
    import concourse.mybir as mybir
    import concourse.tile as tile

    fp32 = mybir.dt.float32
    mmdt = getattr(mybir.dt, mm_dtype)
    AX = mybir.AxisListType.X
    ALL_CORES = [list(range(N_CORES))]
    Act = mybir.ActivationFunctionType

    nc = bacc.Bacc(
        "TRN2",
        target_bir_lowering=False,
        debug=False,
        enable_asserts=False,
        num_devices=N_CORES,
    )

    # et: pre-tiled slice [P, HC*S_PER]; hc: hidden as [P, HC] (chunk j in col j)
    et = nc.dram_tensor("et", [P, HC * S_PER], mmdt, kind="ExternalInput").ap()
    hc = nc.dram_tensor("hc", [P, HC], mmdt, kind="ExternalInput").ap()
    out = nc.dram_tensor("out", [S_PER], fp32, kind="ExternalOutput").ap()

    with tile.TileContext(nc) as tc:
        with (
            tc.tile_pool(name="stream", bufs=6) as stream_pool,
            tc.tile_pool(name="persist", bufs=1) as persist_pool,
            tc.tile_pool(name="small", bufs=1) as small_pool,
            tc.tile_pool(name="psum", bufs=6, space="PSUM") as psum_pool,
            tc.tile_pool(name="dram", bufs=1, space="DRAM") as dram_pool,
        ):
            hid = small_pool.tile([P, HC], mmdt)
            nc.sync.dma_start(out=hid, in_=hc)

            exps_row = persist_pool.tile([1, S_PER], fp32)   # exp(s - m_t)
            attn_row = persist_pool.tile([1, S_PER], fp32)
            pair_row = small_pool.tile([1, 2 * NCH], fp32)   # (m_t, sum_t)
            negm_row = small_pool.tile([1, NCH], fp32)

            cc_in = dram_pool.tile([2 * NCH], fp32)
            cc_out = dram_pool.tile([N_CORES, 2 * NCH], fp32)

            # ---- stream chunks: matmul + incremental softmax stats ----
            for gi, group in enumerate(DMA_GROUPS):
                g0, gsz = OFFS[group[0]], sum(SIZES[c] for c in group)
                etile = stream_pool.tile(
                    [P, HC * gsz], mmdt, tag="et", bufs=6, name=f"et{gi}"
                )
                eng = nc.sync if gi % 2 == 0 else nc.scalar
                eng.dma_start(
                    out=etile, in_=et[:, HC * g0 : HC * (g0 + gsz)]
                )
                # interleave the chunks' PSUM chains so consecutive
                # matmuls target different banks and pipeline
                pss = {
                    c: psum_pool.tile([1, SIZES[c]], fp32, tag="ps", bufs=6,
                                      name=f"ps{c}")
                    for c in group
                }
                for j in range(HC):
                    for c in group:
                        base = HC * (OFFS[c] - g0) + j * SIZES[c]
                        nc.tensor.matmul(
                            pss[c],
                            hid[:, j : j + 1],
                            etile[:, base : base + SIZES[c]],
                            start=(j == 0),
                            stop=(j == HC - 1),
                        )
                for c in group:
                    # stats straight from PSUM; ACT writes exp row + sum
                    nc.vector.reduce_max(pair_row[:, 2 * c : 2 * c + 1], pss[c], axis=AX)
                    nc.vector.tensor_scalar_mul(
                        negm_row[:, c : c + 1], pair_row[:, 2 * c : 2 * c + 1], -1.0
                    )
                    nc.scalar.activation(
                        out=exps_row[:, OFFS[c] : OFFS[c] + SIZES[c]],
                        in_=pss[c],
                        func=Act.Exp,
                        bias=negm_row[:, c : c + 1],
                        scale=1.0,
                        accum_out=pair_row[:, 2 * c + 1 : 2 * c + 2],
                    )

            # ---- one AllGather of all (m, s) pairs; SWDGE (gpsimd) DMA so
            # it doesn't queue behind stream DMAs on the HWDGE rings ----
            nc.gpsimd.dma_start(out=cc_in, in_=pair_row)
            nc.gpsimd.collective_compute(
                "AllGather",
                mybir.AluOpType.bypass,
                replica_groups=ALL_CORES,
                ins=[cc_in.opt()],
                outs=[cc_out.opt()],
            )

            # ---- combine 144 (m, s) pairs -> g, S ----
            NP1 = N_CORES * 2 * NCH             # 288 gathered floats
            NP2 = 0
            row = small_pool.tile([1, NP1 + NP2], fp32)
            nc.scalar.dma_start(out=row, in_=cc_out.rearrange("a b -> (a b)"))
            rowv = row.rearrange("o (k two) -> o two k", two=2)
            g1 = small_pool.tile([1, 1], fp32)
            nc.vector.reduce_max(g1, rowv[:, 0, :], axis=AX)
            negg1 = small_pool.tile([1, 1], fp32)
            nc.vector.tensor_scalar_mul(negg1, g1, -1.0)
            em = small_pool.tile([1, (NP1 + NP2) // 2], fp32)
            nc.scalar.activation(
                out=em, in_=rowv[:, 0, :], func=Act.Exp, bias=negg1, scale=1.0
            )
            terms = small_pool.tile([1, (NP1 + NP2) // 2], fp32)
            nc.vector.tensor_mul(terms, em, rowv[:, 1, :])
            s1 = small_pool.tile([1, 1], fp32)
            nc.vector.reduce_sum(s1, terms, axis=AX)
            rs1 = small_pool.tile([1, 1], fp32)
            nc.vector.reciprocal(rs1, s1)

            # my per-chunk factors f_t = exp(m_t - g) / S
            pairv = pair_row.rearrange("o (k two) -> o two k", two=2)
            ft0 = small_pool.tile([1, NCH], fp32)
            nc.scalar.activation(
                out=ft0, in_=pairv[:, 0, :], func=Act.Exp, bias=negg1, scale=1.0
            )
            ft = small_pool.tile([1, NCH], fp32)
            nc.vector.tensor_scalar_mul(ft, ft0, rs1)

            # ---- rescale, split across Vector and Act engines ----
            for c, size in enumerate(SIZES):
                src = exps_row[:, OFFS[c] : OFFS[c] + size]
                dst = attn_row[:, OFFS[c] : OFFS[c] + size]
                if c % 2 == 0:
                    nc.vector.tensor_scalar_mul(dst, src, ft[:, c : c + 1])
                else:
                    nc.scalar.mul(dst, src, ft[:, c : c + 1])

            half = S_PER // 2
            out2 = out.rearrange("(o s) -> o s", o=1)
            nc.sync.dma_start(out=out2[:, 0:half], in_=attn_row[:, 0:half])
            nc.scalar.dma_start(out=out2[:, half:], in_=attn_row[:, half:])

    nc.compile()
    return nc


def _get_module():
    if "nc" not in _CACHE:
        _CACHE["nc"] = _build_module()
    return _CACHE["nc"]


def _prep_inputs(hidden: np.ndarray, encoder_outputs: np.ndarray):
    hidden = np.asarray(hidden, dtype=np.float32)
    eo = np.asarray(encoder_outputs, dtype=np.float32)
    h16 = hidden.astype(np.float16)
    eo16 = eo.astype(np.float16)
    hcm = np.ascontiguousarray(h16.reshape(HC, P).T)  # [P, HC]
    in_maps = []
    for c in range(N_CORES):
        es = eo16[c * S_PER : (c + 1) * S_PER]  # [S_PER, H]
        blocks = [
            es[OFFS[i] : OFFS[i] + SIZES[i], :]
            .reshape(SIZES[i], HC, P)
            .transpose(2, 1, 0)
            .reshape(P, HC * SIZES[i])
            for i in range(NCH)
        ]
        ets = np.ascontiguousarray(np.concatenate(blocks, axis=1))  # [P, HC*S_PER]
        in_maps.append({"et": ets, "hc": hcm})
    return in_maps


def _run(hidden: np.ndarray, encoder_outputs: np.ndarray, trace: bool = False):
    from concourse.bass_utils import run_bass_kernel_spmd

    nc = _get_module()
    in_maps = _prep_inputs(hidden, encoder_outputs)
    res = run_bass_kernel_spmd(
        nc, in_maps, core_ids=list(range(N_CORES)), trace=trace
    )
    parts = [np.asarray(res.results[c]["out"]).reshape(-1) for c in range(N_CORES)]
    attn = np.concatenate(parts)
    return attn.reshape(1, 1, S_TOTAL).astype(np.float32), res


def kernel(hidden: np.ndarray, encoder_outputs: np.ndarray) -> np.ndarray:
    try:
        out, _ = _run(hidden, encoder_outputs, trace=False)
    except Exception:
        # one retry for transient device/runtime hiccups
        _CACHE.clear()
        out, _ = _run(hidden, encoder_outputs, trace=False)
    return out
